# revision 20
# baseline (speedup 1.0000x reference)
"""KDE-KNN kernel for Trainium2 (8 NeuronCores, SPMD).

Problem: for each of M=8192 points x_i (3-D), among points sharing its group id
(32 groups), find the K=16-th smallest euclidean distance w (self included),
then p_i = pi*w^2/(K-1), with fallback p_i = 1/c_i when the group count c_i < K.

Strategy:
  * Host: sort points by group id (pure layout work). Each group's members are
    then contiguous, so a point's candidate set is one contiguous column window.
  * Device: one bf16 matmul per row-tile computes the NEGATED squared distances
    -d2[m,n] = 2*x_m.x_n - |x_m|^2 - |x_n|^2 with a K=17 contraction: coords +
    norm terms folded into the operands, fp32 precision recovered via a hi/lo
    bf16 split ([A_hi;A_lo;A_hi] @ [B_hi;B_hi;B_lo], only lo*lo dropped), and
    2 extra rows masking cross-group pairs when two small group tails share a
    tile (term -BIG for row/col subgroup mismatch).
  * The 16-th smallest per row is extracted straight out of PSUM with vector
    max8 -> match_replace8 -> max8 (3 passes). Since dim = ni-1 = 2,
    vol = pi*w^2 needs no sqrt:  p = pi/(K-1) * relu(d2_kth).
  * The 8 cores run one shared NEFF; all per-core differences live in the input
    data. Slot 0's operands ship in their own small DMA so the tensor engine
    starts while the main operand DMA is still in flight.
  * Rows whose group has fewer than K members keep the reference fallback
    p = 1/c, applied on the host (their device value is well-defined garbage).
"""

import math

import ml_dtypes
import numpy as np

import concourse.bacc as bacc
import concourse.mybir as mybir
import concourse.tile as tile
from concourse.bass_utils import run_bass_kernel_spmd

M, NI, G = 8192, 3, 32
N_CORES = 8
P = 128  # partitions / rows per tile
KC = 17  # contraction rows: 15 hi/lo + 2 subgroup-mask rows
BIG = 1.0e9  # negated-d2 offset for pad columns / cross-subgroup pairs
NEG_INF = -3.0e38  # match_replace fill
MM_MAX = 512  # max moving free dim per matmul
SLOT_OVERHEAD = 190.0  # per-slot fixed cost in column units (3 DVE op setups)


def _plan_slots(counts, starts):
    """Row tiles (one per (group, 128-row block)), with small tail blocks of
    different groups packed pairwise into one slot (disambiguated by the mask
    rows). Returns a list of slots; each slot is a list of sub-tiles
    (row_start, nrows, row_off, win_start, win_len, col_off)."""
    full, tails = [], []
    for g in range(len(counts)):
        c, s = int(counts[g]), int(starts[g])
        if c == 0:
            continue
        for r0 in range(0, c, P):
            nr = min(P, c - r0)
            (tails if nr <= 64 else full).append((s + r0, nr, s, c))
    slots = [[(rs, nr, 0, ws, wl, 0)] for (rs, nr, ws, wl) in full]
    # pair tails greedily (largest window first) while rows<=P and cols<=1024
    tails.sort(key=lambda t: -t[3])
    while tails:
        rs, nr, ws, wl = tails.pop(0)
        sub = [(rs, nr, 0, ws, wl, 0)]
        for i, (rs2, nr2, ws2, wl2) in enumerate(tails):
            if nr + nr2 <= P and wl + wl2 <= 2 * MM_MAX:
                sub.append((rs2, nr2, nr, ws2, wl2, wl))
                tails.pop(i)
                break
        slots.append(sub)
    return slots


def _slot_cols(slot):
    return sum(s[4] for s in slot)


def _balance(slots, n_cores):
    """Greedy least-loaded assignment; each core's list sorted by descending
    width so slot widths align across cores."""
    order = sorted(range(len(slots)), key=lambda i: -_slot_cols(slots[i]))
    loads = [0.0] * n_cores
    percore = [[] for _ in range(n_cores)]
    for i in order:
        c = loads.index(min(loads))
        percore[c].append(slots[i])
        loads[c] += _slot_cols(slots[i]) + SLOT_OVERHEAD
    return percore


def _split_bf16(a):
    hi = a.astype(ml_dtypes.bfloat16)
    lo = (a - hi.astype(np.float32)).astype(ml_dtypes.bfloat16)
    return hi, lo


def kernel(x: np.ndarray, min_t_idx: np.ndarray, K) -> np.ndarray:
    x = np.asarray(x, dtype=np.float32)
    gid = np.asarray(min_t_idx)
    K = int(K)
    m = x.shape[0]
    assert x.shape == (m, NI) and gid.shape == (m,)

    # ---- host-side layout: sort by group --------------------------------
    perm = np.argsort(gid, kind="stable")
    gp = gid[perm]
    xp = x[perm]
    ngroups = int(gp[-1]) + 1 if m else 0
    counts = np.bincount(gp, minlength=ngroups)
    starts = np.concatenate([[0], np.cumsum(counts)[:-1]])

    sq = np.sum(xp * xp, axis=1, dtype=np.float32)
    # lhsT source rows: [x0, x1, x2, -sq, -1]; rhs source rows: [2x0, 2x1, 2x2, 1, sq]
    A = np.empty((5, m), dtype=np.float32)
    A[0:3] = xp.T
    A[3] = -sq
    A[4] = -1.0
    B = np.empty((5, m), dtype=np.float32)
    B[0:3] = 2.0 * xp.T
    B[3] = 1.0
    B[4] = sq
    A_hi, A_lo = _split_bf16(A)
    B_hi, B_lo = _split_bf16(B)
    A15 = np.vstack([A_hi, A_lo, A_hi])  # [15, m] bf16
    B15 = np.vstack([B_hi, B_hi, B_lo])

    percore = _balance(_plan_slots(counts, starts), N_CORES)
    T = max(len(sl) for sl in percore)
    # per-slot-index width = max over cores (uniform program across cores)
    W = [
        max(_slot_cols(sl[t]) if t < len(sl) else 8 for sl in percore)
        for t in range(T)
    ]
    W = [max(8, (w + 3) & ~3) for w in W]
    offs = np.concatenate([[0], np.cumsum(W)]).astype(int)
    SW = int(offs[-1])
    # uniform per-slot-index matmul column chunking (shared by all cores):
    # chunks at sub-tile boundaries would differ per core, so chunk at fixed
    # MM_MAX boundaries instead and let the mask rows handle any mixing.
    mm_chunks = [
        [(c0, min(MM_MAX, W[t] - c0)) for c0 in range(0, W[t], MM_MAX)]
        for t in range(T)
    ]

    # ---- per-core input marshaling --------------------------------------
    # one contiguous [KC, P + W[t]] operand block per slot -> one DMA per slot
    # (separate dma_starts land on separate HW DMA engines and each slot's
    # matmul depends only on its own block)
    big_bf = ml_dtypes.bfloat16(BIG)
    nbig_bf = ml_dtypes.bfloat16(-BIG)
    blk_off = np.concatenate([[0], np.cumsum([P + w for w in W])]).astype(int)
    AB = int(blk_off[-1])
    in_maps = []
    for sl in percore:
        ab = np.zeros((KC, AB), dtype=ml_dtypes.bfloat16)
        for t, slot in enumerate(sl):
            lhs = ab[:, int(blk_off[t]) : int(blk_off[t]) + P]
            rhs = ab[:, int(blk_off[t]) + P : int(blk_off[t + 1])]
            rhs[4, :] = big_bf  # pad columns pair with lhsT row4 = -1 -> -BIG
            for si, (rs, nr, ro, ws, wl, co) in enumerate(slot):
                lhs[:15, ro : ro + nr] = A15[:, rs : rs + nr]
                # mask rows: row15 = -BIG for subgroup b rows; row16 = -BIG for a
                lhs[15, ro : ro + nr] = nbig_bf if si else 0.0
                lhs[16, ro : ro + nr] = 0.0 if si else nbig_bf
                rhs[:15, co : co + wl] = B15[:, ws : ws + wl]
                rhs[15, co : co + wl] = 0.0 if si else 1.0  # (1-cb)
                rhs[16, co : co + wl] = 1.0 if si else 0.0  # cb
        for t in range(len(sl), T):
            ab[4, int(blk_off[t]) + P : int(blk_off[t + 1])] = big_bf
        in_maps.append({"ab": ab})

    # ---- build the device program (shared by all cores) -----------------
    nc = bacc.Bacc("TRN2", target_bir_lowering=False, debug=False, num_devices=N_CORES)
    ab_d = nc.dram_tensor("ab", [KC, AB], mybir.dt.bfloat16, kind="ExternalInput")
    out_d = nc.dram_tensor("out", [P, T], mybir.dt.float32, kind="ExternalOutput")

    rounds = max(1, (K + 7) // 8)  # max8 rounds; match_replace between them
    last_col = (K - 1) - 8 * (rounds - 1)
    scale = -math.pi / max(K - 1, 1)

    with tile.TileContext(nc) as tc:
        with (
            tc.tile_pool(name="io", bufs=1) as io_pool,
            tc.tile_pool(name="small", bufs=4) as small_pool,
            tc.tile_pool(name="psum", bufs=6, space="PSUM") as psum_pool,
            tc.tile_pool(name="psum2", bufs=1, space="PSUM") as psum2_pool,
        ):
            ab_sb = io_pool.tile([KC, AB], mybir.dt.bfloat16)
            m8_all = io_pool.tile([P, T, 8], mybir.dt.float32)
            out_sb = io_pool.tile([P, T], mybir.dt.float32)
            for t in range(T):
                # alternate between the two HWDGE queues (SP + Activation)
                eng = nc.sync if t % 2 == 0 else nc.scalar
                eng.dma_start(
                    ab_sb[:, int(blk_off[t]) : int(blk_off[t + 1])],
                    ab_d[:, int(blk_off[t]) : int(blk_off[t + 1])],
                )

            for t in range(T):
                lhs_t = ab_sb[:, int(blk_off[t]) : int(blk_off[t]) + P]
                rhs_t = ab_sb[:, int(blk_off[t]) + P : int(blk_off[t + 1])]
                pool_t = psum2_pool if W[t] > MM_MAX else psum_pool
                ps = pool_t.tile(
                    [P, W[t]], mybir.dt.float32,
                    tag="ps2" if W[t] > MM_MAX else "ps",
                )
                for ci, (c0, cl) in enumerate(mm_chunks[t]):
                    nc.tensor.matmul(
                        ps[:, c0 : c0 + cl],
                        lhs_t,
                        rhs_t[:, c0 : c0 + cl],
                        start=True,
                        stop=True,
                    )
                m8 = small_pool.tile([P, 8], mybir.dt.float32, tag="m8")
                for _ in range(rounds - 1):
                    nc.vector.max(out=m8[:], in_=ps[:])
                    nc.vector.match_replace(
                        out=ps[:], in_to_replace=m8[:], in_values=ps[:],
                        imm_value=NEG_INF,
                    )
                nc.vector.max(out=m8_all[:, t, :], in_=ps[:])

            # p = (pi/(K-1)) * relu(d2_kth); m8 holds -d2 so scale<0 then max 0
            nc.vector.tensor_scalar(
                out_sb[:],
                m8_all[:, :, last_col],
                float(scale),
                0.0,
                op0=mybir.AluOpType.mult,
                op1=mybir.AluOpType.max,
            )
            nc.sync.dma_start(out_d[:], out_sb[:])

    nc.compile()

    # If BASS_TRACE is set but this image's antenv lacks axon_hooks, inject a
    # None-returning stub so run_bass_kernel_spmd degrades to untraced.
    try:
        import antenv.axon_hooks  # noqa: F401
    except ImportError:
        import sys
        import types

        _m = types.ModuleType("antenv.axon_hooks")
        _m.get_axon_ntff_profile_hook = lambda: None
        _m.set_axon_ntff_profile_hook = lambda h: None
        sys.modules["antenv.axon_hooks"] = _m

    res = run_bass_kernel_spmd(nc, in_maps, core_ids=list(range(N_CORES)))
    global LAST_RESULTS
    LAST_RESULTS = res

    # ---- gather / unshard ----------------------------------------------
    p_perm = np.empty((m,), dtype=np.float32)
    for core, sl in enumerate(percore):
        o = res.results[core]["out"]  # [P, T]: column t holds slot t's rows
        for t, slot in enumerate(sl):
            for (rs, nr, ro, _ws, _wl, _co) in slot:
                p_perm[rs : rs + nr] = o[ro : ro + nr, t]
    # reference fallback for rows whose group is smaller than K: p = 1/c
    crow = counts[gp]
    small = crow < K
    if small.any():
        p_perm[small] = (
            np.float32(1.0) / crow[small].astype(np.float32)
        ).astype(np.float32)
    p = np.empty((m,), dtype=np.float32)
    p[perm] = p_perm
    return p


# revision 23
# speedup vs baseline: 1.0663x; 1.0663x over previous
"""KDE-KNN kernel for Trainium2 (8 NeuronCores, SPMD).

Problem: for each of M=8192 points x_i (3-D), among points sharing its group id
(32 groups), find the K=16-th smallest euclidean distance w (self included),
then p_i = pi*w^2/(K-1), with fallback p_i = 1/c_i when the group count c_i < K.

Strategy:
  * Host: sort points by group id (pure layout work). Each group's members are
    then contiguous, so a point's candidate set is one contiguous column window.
  * Device: one bf16 matmul per row-tile computes the NEGATED squared distances
    -d2[m,n] = 2*x_m.x_n - |x_m|^2 - |x_n|^2 with a K=17 contraction: coords +
    norm terms folded into the operands, fp32 precision recovered via a hi/lo
    bf16 split ([A_hi;A_lo;A_hi] @ [B_hi;B_hi;B_lo], only lo*lo dropped), and
    2 extra rows masking cross-group pairs when two small group tails share a
    tile (term -BIG for row/col subgroup mismatch).
  * The 16-th smallest per row is extracted straight out of PSUM with vector
    max8 -> match_replace8 -> max8 (3 passes). Since dim = ni-1 = 2,
    vol = pi*w^2 needs no sqrt:  p = pi/(K-1) * relu(d2_kth).
  * The 8 cores run one shared NEFF; all per-core differences live in the input
    data. Slot 0's operands ship in their own small DMA so the tensor engine
    starts while the main operand DMA is still in flight.
  * Rows whose group has fewer than K members keep the reference fallback
    p = 1/c, applied on the host (their device value is well-defined garbage).
"""

import math

import ml_dtypes
import numpy as np

import concourse.bacc as bacc
import concourse.mybir as mybir
import concourse.tile as tile
from concourse.bass_utils import run_bass_kernel_spmd

M, NI, G = 8192, 3, 32
N_CORES = 8
P = 128  # partitions / rows per tile
KC = 17  # contraction rows: 15 hi/lo + 2 subgroup-mask rows
BIG = 1.0e9  # negated-d2 offset for pad columns / cross-subgroup pairs
NEG_INF = -3.0e38  # match_replace fill
MM_MAX = 512  # max moving free dim per matmul
SLOT_OVERHEAD = 190.0  # per-slot fixed cost in column units (3 DVE op setups)


def _plan_slots(counts, starts):
    """Row tiles (one per (group, 128-row block)), with small tail blocks of
    different groups packed pairwise into one slot (disambiguated by the mask
    rows). Returns a list of slots; each slot is a list of sub-tiles
    (row_start, nrows, row_off, win_start, win_len, col_off)."""
    full, tails = [], []
    for g in range(len(counts)):
        c, s = int(counts[g]), int(starts[g])
        if c == 0:
            continue
        for r0 in range(0, c, P):
            nr = min(P, c - r0)
            (tails if nr <= 64 else full).append((s + r0, nr, s, c))
    slots = [[(rs, nr, 0, ws, wl, 0)] for (rs, nr, ws, wl) in full]
    # pair tails greedily (largest window first) while rows<=P and cols<=1024
    tails.sort(key=lambda t: -t[3])
    while tails:
        rs, nr, ws, wl = tails.pop(0)
        sub = [(rs, nr, 0, ws, wl, 0)]
        for i, (rs2, nr2, ws2, wl2) in enumerate(tails):
            if nr + nr2 <= P and wl + wl2 <= 2 * MM_MAX:
                sub.append((rs2, nr2, nr, ws2, wl2, wl))
                tails.pop(i)
                break
        slots.append(sub)
    return slots


def _slot_cols(slot):
    return sum(s[4] for s in slot)


def _balance(slots, n_cores):
    """Greedy least-loaded assignment; each core's list sorted by ascending
    width (so widths align across cores, and the first slot's DMA + matmul
    are the cheapest -> earliest vector start)."""
    order = sorted(range(len(slots)), key=lambda i: -_slot_cols(slots[i]))
    loads = [0.0] * n_cores
    percore = [[] for _ in range(n_cores)]
    for i in order:
        c = loads.index(min(loads))
        percore[c].append(slots[i])
        loads[c] += _slot_cols(slots[i]) + SLOT_OVERHEAD
    for tl in percore:
        tl.reverse()
    return percore


def _split_bf16(a):
    hi = a.astype(ml_dtypes.bfloat16)
    lo = (a - hi.astype(np.float32)).astype(ml_dtypes.bfloat16)
    return hi, lo


def kernel(x: np.ndarray, min_t_idx: np.ndarray, K) -> np.ndarray:
    x = np.asarray(x, dtype=np.float32)
    gid = np.asarray(min_t_idx)
    K = int(K)
    m = x.shape[0]
    assert x.shape == (m, NI) and gid.shape == (m,)

    # ---- host-side layout: sort by group --------------------------------
    perm = np.argsort(gid, kind="stable")
    gp = gid[perm]
    xp = x[perm]
    ngroups = int(gp[-1]) + 1 if m else 0
    counts = np.bincount(gp, minlength=ngroups)
    starts = np.concatenate([[0], np.cumsum(counts)[:-1]])

    sq = np.sum(xp * xp, axis=1, dtype=np.float32)
    # lhsT source rows: [x0, x1, x2, -sq, -1]; rhs source rows: [2x0, 2x1, 2x2, 1, sq]
    A = np.empty((5, m), dtype=np.float32)
    A[0:3] = xp.T
    A[3] = -sq
    A[4] = -1.0
    B = np.empty((5, m), dtype=np.float32)
    B[0:3] = 2.0 * xp.T
    B[3] = 1.0
    B[4] = sq
    A_hi, A_lo = _split_bf16(A)
    B_hi, B_lo = _split_bf16(B)
    A15 = np.vstack([A_hi, A_lo, A_hi])  # [15, m] bf16
    B15 = np.vstack([B_hi, B_hi, B_lo])

    percore = _balance(_plan_slots(counts, starts), N_CORES)
    T = max(len(sl) for sl in percore)
    # per-slot-index width = max over cores (uniform program across cores)
    W = [
        max(_slot_cols(sl[t]) if t < len(sl) else 8 for sl in percore)
        for t in range(T)
    ]
    W = [max(8, (w + 3) & ~3) for w in W]
    offs = np.concatenate([[0], np.cumsum(W)]).astype(int)
    SW = int(offs[-1])
    # uniform per-slot-index matmul column chunking (shared by all cores):
    # chunks at sub-tile boundaries would differ per core, so chunk at fixed
    # MM_MAX boundaries instead and let the mask rows handle any mixing.
    mm_chunks = [
        [(c0, min(MM_MAX, W[t] - c0)) for c0 in range(0, W[t], MM_MAX)]
        for t in range(T)
    ]

    # ---- per-core input marshaling --------------------------------------
    # one contiguous [KC, P + W[t]] operand block per slot -> one DMA per slot
    # (separate dma_starts land on separate HW DMA engines and each slot's
    # matmul depends only on its own block)
    big_bf = ml_dtypes.bfloat16(BIG)
    nbig_bf = ml_dtypes.bfloat16(-BIG)
    blk_off = np.concatenate([[0], np.cumsum([P + w for w in W])]).astype(int)
    AB = int(blk_off[-1])
    in_maps = []
    for sl in percore:
        ab = np.zeros((KC, AB), dtype=ml_dtypes.bfloat16)
        for t, slot in enumerate(sl):
            lhs = ab[:, int(blk_off[t]) : int(blk_off[t]) + P]
            rhs = ab[:, int(blk_off[t]) + P : int(blk_off[t + 1])]
            rhs[4, :] = big_bf  # pad columns pair with lhsT row4 = -1 -> -BIG
            for si, (rs, nr, ro, ws, wl, co) in enumerate(slot):
                lhs[:15, ro : ro + nr] = A15[:, rs : rs + nr]
                # mask rows: row15 = -BIG for subgroup b rows; row16 = -BIG for a
                lhs[15, ro : ro + nr] = nbig_bf if si else 0.0
                lhs[16, ro : ro + nr] = 0.0 if si else nbig_bf
                rhs[:15, co : co + wl] = B15[:, ws : ws + wl]
                rhs[15, co : co + wl] = 0.0 if si else 1.0  # (1-cb)
                rhs[16, co : co + wl] = 1.0 if si else 0.0  # cb
        for t in range(len(sl), T):
            ab[4, int(blk_off[t]) + P : int(blk_off[t + 1])] = big_bf
        in_maps.append({"ab": ab})

    # ---- build the device program (shared by all cores) -----------------
    nc = bacc.Bacc("TRN2", target_bir_lowering=False, debug=False, num_devices=N_CORES)
    ab_d = nc.dram_tensor("ab", [KC, AB], mybir.dt.bfloat16, kind="ExternalInput")
    out_d = nc.dram_tensor("out", [P, T], mybir.dt.float32, kind="ExternalOutput")

    rounds = max(1, (K + 7) // 8)  # max8 rounds; match_replace between them
    last_col = (K - 1) - 8 * (rounds - 1)
    scale = -math.pi / max(K - 1, 1)

    with tile.TileContext(nc) as tc:
        with (
            tc.tile_pool(name="io", bufs=1) as io_pool,
            tc.tile_pool(name="small", bufs=4) as small_pool,
            tc.tile_pool(name="psum", bufs=6, space="PSUM") as psum_pool,
            tc.tile_pool(name="psum2", bufs=1, space="PSUM") as psum2_pool,
        ):
            ab_sb = io_pool.tile([KC, AB], mybir.dt.bfloat16)
            m8_all = io_pool.tile([P, T, 8], mybir.dt.float32)
            out_sb = io_pool.tile([P, T], mybir.dt.float32)
            for t in range(T):
                nc.sync.dma_start(
                    ab_sb[:, int(blk_off[t]) : int(blk_off[t + 1])],
                    ab_d[:, int(blk_off[t]) : int(blk_off[t + 1])],
                )

            for t in range(T):
                lhs_t = ab_sb[:, int(blk_off[t]) : int(blk_off[t]) + P]
                rhs_t = ab_sb[:, int(blk_off[t]) + P : int(blk_off[t + 1])]
                pool_t = psum2_pool if W[t] > MM_MAX else psum_pool
                ps = pool_t.tile(
                    [P, W[t]], mybir.dt.float32,
                    tag="ps2" if W[t] > MM_MAX else "ps",
                )
                for ci, (c0, cl) in enumerate(mm_chunks[t]):
                    nc.tensor.matmul(
                        ps[:, c0 : c0 + cl],
                        lhs_t,
                        rhs_t[:, c0 : c0 + cl],
                        start=True,
                        stop=True,
                    )
                m8 = small_pool.tile([P, 8], mybir.dt.float32, tag="m8")
                for _ in range(rounds - 1):
                    nc.vector.max(out=m8[:], in_=ps[:])
                    nc.vector.match_replace(
                        out=ps[:], in_to_replace=m8[:], in_values=ps[:],
                        imm_value=NEG_INF,
                    )
                nc.vector.max(out=m8_all[:, t, :], in_=ps[:])

            # p = (pi/(K-1)) * relu(d2_kth); m8 holds -d2 so scale<0 then max 0.
            # Two halves so the first out-DMA overlaps the last slots' work.
            th = T // 2
            for lo, hi in ((0, th), (th, T)):
                nc.vector.tensor_scalar(
                    out_sb[:, lo:hi],
                    m8_all[:, lo:hi, last_col],
                    float(scale),
                    0.0,
                    op0=mybir.AluOpType.mult,
                    op1=mybir.AluOpType.max,
                )
                nc.sync.dma_start(out_d[:, lo:hi], out_sb[:, lo:hi])

    nc.compile()

    # If BASS_TRACE is set but this image's antenv lacks axon_hooks, inject a
    # None-returning stub so run_bass_kernel_spmd degrades to untraced.
    try:
        import antenv.axon_hooks  # noqa: F401
    except ImportError:
        import sys
        import types

        _m = types.ModuleType("antenv.axon_hooks")
        _m.get_axon_ntff_profile_hook = lambda: None
        _m.set_axon_ntff_profile_hook = lambda h: None
        sys.modules["antenv.axon_hooks"] = _m

    res = run_bass_kernel_spmd(nc, in_maps, core_ids=list(range(N_CORES)))
    global LAST_RESULTS
    LAST_RESULTS = res

    # ---- gather / unshard ----------------------------------------------
    p_perm = np.empty((m,), dtype=np.float32)
    for core, sl in enumerate(percore):
        o = res.results[core]["out"]  # [P, T]: column t holds slot t's rows
        for t, slot in enumerate(sl):
            for (rs, nr, ro, _ws, _wl, _co) in slot:
                p_perm[rs : rs + nr] = o[ro : ro + nr, t]
    # reference fallback for rows whose group is smaller than K: p = 1/c
    crow = counts[gp]
    small = crow < K
    if small.any():
        p_perm[small] = (
            np.float32(1.0) / crow[small].astype(np.float32)
        ).astype(np.float32)
    p = np.empty((m,), dtype=np.float32)
    p[perm] = p_perm
    return p
